# revision 53
# baseline (speedup 1.0000x reference)
"""Trainium2 Bass kernel for GroundwaterModel Jacobi pseudo-timestepping.

100 Jacobi steps of -div(exp(u) grad p) = f on [1024,1024], sharded row-wise
across 8 NeuronCores.  Communication-avoiding deep-halo scheme: each core
steps its 128 main rows PLUS a 128-row halo region (64 rows from each
neighbor) redundantly, so only ONE AllGather (after update 65) is needed for
the whole run instead of one per step.

Math: symmetrized update q = sqrt(D) p.  Per step, per region:
  q'[i,k] = u1[i+1,k] + u2[i-1,k] + y3[i,k] + y4[i,k] + C[i,k]
  u1 = BXD*q, u2 = BXU*q, y3[k] = BYR[k]*q[k+1], y4[k] = (BYR*q)[k-1]
All coefficient fields are host-precomputed (fp64) and shipped as fp16.
Engine split per step: ACT copies q (PSUM fp32 -> SBUF fp16); DVE (+Pool)
computes the 4 products + y3+y4 add in fp16 2x mode; PE accumulates all
shifts/adds as fp16 matmuls into PSUM (partition shifts = shift matrices,
cross-region couplings = selection matrices, plain adds = identity).

Tile layout: [128, 2, 1024] (region 0 = main rows, region 1 = halo rows;
region cols 1..1022 hold interior grid cols j=1..1022, cols 0/1023 are zero
padding so shifted free-dim reads never go out of bounds).
"""

import numpy as np

GRID = 1024
NCORES = 8
P = 128            # rows per core = SBUF partitions
W = GRID - 2       # interior columns j=1..1022
K = 64             # halo depth per side
RST = 1024         # region stride (free dim)

# scheduling knobs (sweepable; best combo found by TimelineSim sweep)
OPTS = {
    "pool": "none",        # none | main_y | halo_y : products on Pool engine
    "a1": "pe",            # dve | pe : where y3+y4 is summed, per region
    "cross_late": True,    # emit Xa/Xb after the other main-bank matmuls
    "halo_first": False,   # PE computes main banks before halo banks
    "copy_dve_halo": False,  # both PSUM->SBUF copies on ACT (halo mid-step)
    "hx_last": False,      # emit halo-region cross matmuls after main phase
    "um_psum": False,      # main u1/u2 products read PSUM (skip ACT latency)
    # timing-surgery probes (break correctness; TimelineSim only)
    "no_copy": False,      # skip ACT copies
    "no_cross": False,     # skip cross matmuls
    "no_halo": False,      # skip all halo-region work
    "no_products": False,  # skip DVE products
}

_cached = {}


def _host_inputs(u, f, n_cores, time_steps):
    N = u.shape[0]
    h = 1.0 / (N - 1)
    u64 = np.asarray(u, np.float64)
    f64 = np.asarray(f, np.float64)
    EU = np.exp(u64)
    EU_xm = np.vstack([EU[:1], EU[:-1]])
    EU_ym = np.hstack([EU[:, :1], EU[:, :-1]])
    D = 2.0 * EU + EU_xm + EU_ym
    S = np.sqrt(D)
    RS = 1.0 / S

    # x-direction product coefficients (global, j-indexed)
    BXU = np.empty((N, N))          # u2 coeff: from-below source at own row
    BXU[:-1] = EU[:-1] * RS[:-1] * RS[1:]
    BXU[-1] = EU[-1] * RS[-1] * RS[-1]      # Neumann self (core 7 corner)
    BXD = np.empty((N, N))          # u1 coeff: from-above source at own row
    BXD[1:] = EU[:-1] * RS[1:] * RS[:-1]
    BXD[0] = EU[0] * RS[0] * RS[0]          # Neumann self (core 0 corner)

    xs = np.arange(N) * h
    h2f = (h * h) * f64
    Cj = h2f * RS                   # constant term (j-indexed)
    Cj[:, 1] += EU[:, 0] * xs * RS[:, 1]            # Dirichlet fold j=1
    Cj[:, N - 2] += EU[:, N - 2] * (1.0 - xs) * RS[:, N - 2]
    C0j = h2f * RS                  # first-step constant (p0 BCs are zero)

    # y-direction coefficient, tile-k indexed (k = j-1, 0..W-1)
    BYR = np.zeros((N, W))
    BYR[:, :W - 1] = EU[:, 1:N - 2] * RS[:, 1:N - 2] * RS[:, 2:N - 1]

    def kslice(Gj):                 # j-indexed [N,N] -> k-indexed [N,W]
        return Gj[:, 1:N - 1]

    in_maps = []
    for c in range(n_cores):
        r0 = c * P
        rows_m = np.arange(r0, r0 + P)
        rows_h = np.concatenate([
            np.clip(np.arange(r0 - K, r0), 0, N - 1),
            np.clip(np.arange(r0 + P, r0 + P + K), 0, N - 1),
        ])

        def pack(Gk):               # k-indexed [N,W] -> [P, 2*RST] fp16
            t = np.zeros((P, 2, RST), np.float32)
            t[:, 0, 1:1 + W] = Gk[rows_m]
            t[:, 1, 1:1 + W] = Gk[rows_h]
            return t.reshape(P, 2 * RST).astype(np.float16)

        bxd = pack(kslice(BXD))
        bxu = pack(kslice(BXU))
        byr = pack(BYR)
        cc = pack(kslice(Cj))
        c0 = pack(kslice(C0j))
        rsm = RS[rows_m, 1:N - 1].astype(np.float32)

        # stationary matrices (math convention out = M @ src; ship M.T)
        supm = np.zeros((P, P), np.float32)
        for i in range(P - 1):
            supm[i, i + 1] = 1.0
        if c == 0:
            supm[0, 0] = 1.0
        sdnm = np.zeros((P, P), np.float32)
        for i in range(1, P):
            sdnm[i, i - 1] = 1.0
        if c == n_cores - 1:
            sdnm[P - 1, P - 1] = 1.0
        xa = np.zeros((P, P), np.float32)
        if c < n_cores - 1:
            xa[P - 1, K] = 1.0      # main row 127 <- u1h[64]
        xb = np.zeros((P, P), np.float32)
        if c > 0:
            xb[0, K - 1] = 1.0      # main row 0 <- u2h[63]
        suph = np.zeros((P, P), np.float32)
        for m in range(P - 1):
            if m != K - 1:
                suph[m, m + 1] = 1.0
        sdnh = np.zeros((P, P), np.float32)
        for m in range(1, P):
            if m != K:
                sdnh[m, m - 1] = 1.0
        xc = np.zeros((P, P), np.float32)
        if c > 0:
            xc[K - 1, 0] = 1.0      # halo row 63 <- u1m[0]
        xd = np.zeros((P, P), np.float32)
        if c < n_cores - 1:
            xd[K, P - 1] = 1.0      # halo row 64 <- u2m[127]
        eye = np.eye(P, dtype=np.float32)
        stat = np.concatenate(
            [M.T for M in [supm, sdnm, xa, xb, suph, sdnh, xc, xd, eye]],
            axis=1).astype(np.float16)

        es = np.zeros((n_cores, P, P), np.float32)
        if c > 0:
            for m in range(K):
                es[c - 1][m, K + m] = 1.0       # below-halo <- left block
        if c < n_cores - 1:
            for m in range(K):
                es[c + 1][K + m, m] = 1.0       # above-halo <- right block
        esT = np.concatenate([es[b].T for b in range(n_cores)],
                             axis=1).astype(np.float16)

        in_maps.append({
            "bxd": bxd, "bxu": bxu, "byr": byr, "cc": cc, "c0": c0,
            "rsm": rsm, "stat": stat, "es": esT,
        })
    return in_maps


def _build(n_cores, time_steps, nx, ny):
    import concourse.bass as bass
    import concourse.bacc as bacc
    import concourse.mybir as mybir
    from concourse.tile import TileContext

    f32 = mybir.dt.float32
    f16 = mybir.dt.float16

    nc = bacc.Bacc(
        "TRN2",
        target_bir_lowering=False,
        debug=False,
        num_devices=n_cores,
    )
    dp = nc.declare_dram_parameter
    bxd_d = dp("bxd", [P, 2 * RST], f16, isOutput=False)
    bxu_d = dp("bxu", [P, 2 * RST], f16, isOutput=False)
    byr_d = dp("byr", [P, 2 * RST], f16, isOutput=False)
    cc_d = dp("cc", [P, 2 * RST], f16, isOutput=False)
    c0_d = dp("c0", [P, 2 * RST], f16, isOutput=False)
    rsm_d = dp("rsm", [P, W], f32, isOutput=False)
    stat_d = dp("stat", [P, 9 * P], f16, isOutput=False)
    es_d = dp("es", [P, n_cores * P], f16, isOutput=False)
    pout_d = dp("pout", [P, W], f32, isOutput=True)

    # stationary index helpers (order in the stat param)
    SUPM, SDNM, XA, XB, SUPH, SDNH, XC, XD, EYE = range(9)

    BANKS = [(1, 512), (512, 1023)]     # region-relative matmul col ranges
    ex_ts = set(range(K + 1, time_steps, K))    # exchange after these updates

    with TileContext(nc) as tc:
        with (
            tc.tile_pool(name="coef", bufs=1) as coef,
            tc.tile_pool(name="qp", bufs=2, space="PSUM") as qp,
            tc.tile_pool(name="dramp", bufs=1, space="DRAM") as dramp,
        ):
            bxd = coef.tile([P, 2, RST], f16, name="bxd_t")
            bxu = coef.tile([P, 2, RST], f16, name="bxu_t")
            byr = coef.tile([P, 2, RST], f16, name="byr_t")
            cc = coef.tile([P, 2, RST], f16, name="cc_t")
            c0 = coef.tile([P, 2, RST], f16, name="c0_t")
            rsm = coef.tile([P, W], f32, name="rsm_t")
            stat = coef.tile([P, 9 * P], f16, name="stat_t")
            es = coef.tile([P, n_cores * P], f16, name="es_t")
            nc.sync.dma_start(out=bxd[:, :, :], in_=bxd_d[:, :])
            nc.sync.dma_start(out=bxu[:, :, :], in_=bxu_d[:, :])
            nc.sync.dma_start(out=byr[:, :, :], in_=byr_d[:, :])
            nc.sync.dma_start(out=cc[:, :, :], in_=cc_d[:, :])
            nc.sync.dma_start(out=c0[:, :, :], in_=c0_d[:, :])
            nc.sync.dma_start(out=rsm[:, :], in_=rsm_d[:, :])
            nc.sync.dma_start(out=stat[:, :], in_=stat_d[:, :])
            nc.sync.dma_start(out=es[:, :], in_=es_d[:, :])

            def st(i):
                return stat[:, i * P:(i + 1) * P]

            # double-buffered work tiles (manual a/b rotation)
            q16 = [coef.tile([P, 2, RST], f16, name=f"q16_{i}")
                   for i in range(2)]
            u1 = [coef.tile([P, 2, RST], f16, name=f"u1_{i}")
                  for i in range(2)]
            u2 = [coef.tile([P, 2, RST], f16, name=f"u2_{i}")
                  for i in range(2)]
            y3 = [coef.tile([P, 2, RST], f16, name=f"y3_{i}")
                  for i in range(2)]
            pp = [coef.tile([P, 2, RST], f16, name=f"pp_{i}")
                  for i in range(2)]
            a1 = [coef.tile([P, 2, RST], f16, name=f"a1_{i}")
                  for i in range(2)]
            V = nc.vector
            PLm = nc.gpsimd if OPTS["pool"] == "main_y" else nc.vector
            PLh = nc.gpsimd if OPTS["pool"] == "halo_y" else nc.vector
            for i in range(2):
                # zero the pad cols read by shifted accesses
                V.memset(pp[i][:, :, 0:1], 0.0)     # y4 shift reads col 0
                V.memset(q16[i][:, :, 1023:1024], 0.0)  # y3 shift reads 1023

            mm = nc.tensor.matmul
            AC = nc.scalar

            ps = None
            ps_prev = None
            for t in range(2, time_steps + 1):
                cur = t % 2
                prv = 1 - cur
                q = c0 if t == 2 else q16[prv]
                # main-region q source for the u-products: previous step's
                # PSUM directly (fp32, skips the ACT-copy latency) or q16
                if OPTS["um_psum"] and t > 2:
                    qum = ps_prev[:, 1:1023]
                else:
                    qum = q[:, 0, 1:1023]

                # ---- products (fp16; region-split for pipelining) ----
                def products(r, PL, full):
                    qu = qum if r == 0 else q[:, r, 1:1023]
                    V.tensor_mul(u1[cur][:, r, 1:1023], bxd[:, r, 1:1023],
                                 qu)
                    V.tensor_mul(u2[cur][:, r, 1:1023], bxu[:, r, 1:1023],
                                 qu)
                    if not full:
                        return
                    PL.tensor_mul(y3[cur][:, r, 1:1023],
                                  byr[:, r, 1:1023], q[:, r, 2:1024])
                    PL.tensor_mul(pp[cur][:, r, 1:1023],
                                  byr[:, r, 1:1023], q[:, r, 1:1023])
                    if (OPTS["a1"] == "dve"
                            or (OPTS["a1"] == "hpool" and r == 1)):
                        eng = (nc.gpsimd if OPTS["a1"] == "hpool" else V)
                        eng.tensor_add(a1[cur][:, r, 1:1023],
                                       y3[cur][:, r, 1:1023],
                                       pp[cur][:, r, 0:1022])

                # halo-region matmuls (and their y-products) are not needed
                # on the final update — only the halo u-products feeding the
                # main crosses are
                halo_full = (t < time_steps and t not in ex_ts
                             and not OPTS["no_halo"])
                if not OPTS["no_products"]:
                    if OPTS["no_halo"]:
                        products(0, PLm, True)
                    elif OPTS["halo_first"]:
                        products(1, PLh, halo_full)
                        products(0, PLm, True)
                    else:
                        products(0, PLm, True)
                        products(1, PLh, halo_full)

                # ---- PE accumulation ----
                # separate PSUM tiles per region: dependency tracking on a
                # PSUM tile is epoch-granular, so a shared tile would make
                # every reader wait for BOTH regions' matmul groups
                psnm = qp.tile([P, RST], f32, tag="psm", name=f"psm_{t}")
                psnh = (qp.tile([P, RST], f32, tag="psh", name=f"psh_{t}")
                        if (halo_full or t in ex_ts) else None)
                psr = {0: psnm, 1: psnh}

                def crosses(r, X1, X2):
                    o = 1 - r
                    pst = psr[r]
                    for lo, hi in BANKS:
                        mm(pst[:, lo:hi], st(X1), u1[cur][:, o, lo:hi],
                           start=False, stop=False)
                        mm(pst[:, lo:hi], st(X2), u2[cur][:, o, lo:hi],
                           start=False, stop=True)

                def region(r, S_UP, S_DN, X1, X2, defer_cross=False):
                    # X1/X2: cross matmuls reading the OTHER region of u1/u2
                    # term-major emission: late-ready inputs fold last
                    o = 1 - r
                    pst = psr[r]
                    a1_fold = (OPTS["a1"] == "dve"
                               or (OPTS["a1"] == "hpool" and r == 1))
                    for lo, hi in BANKS:
                        mm(pst[:, lo:hi], st(EYE), cc[:, r, lo:hi],
                           start=True, stop=False)
                    for lo, hi in BANKS:
                        mm(pst[:, lo:hi], st(S_UP), u1[cur][:, r, lo:hi],
                           start=False, stop=False)
                    for lo, hi in BANKS:
                        mm(pst[:, lo:hi], st(S_DN), u2[cur][:, r, lo:hi],
                           start=False, stop=False)
                    if a1_fold:
                        for lo, hi in BANKS:
                            mm(pst[:, lo:hi], st(EYE),
                               a1[cur][:, r, lo:hi],
                               start=False, stop=False)
                    else:
                        for lo, hi in BANKS:
                            mm(pst[:, lo:hi], st(EYE),
                               y3[cur][:, r, lo:hi],
                               start=False, stop=False)
                        for lo, hi in BANKS:
                            mm(pst[:, lo:hi], st(EYE),
                               pp[cur][:, r, lo - 1:hi - 1],
                               start=False, stop=False)
                    if OPTS["no_cross"]:
                        for lo, hi in BANKS:
                            mm(pst[:, lo:hi], st(EYE), cc[:, r, lo:hi],
                               start=False, stop=True)
                    elif not defer_cross:
                        crosses(r, X1, X2)

                if OPTS["halo_first"]:
                    if halo_full:
                        region(1, SUPH, SDNH, XC, XD,
                               defer_cross=OPTS["hx_last"])
                    region(0, SUPM, SDNM, XA, XB)
                    if halo_full and OPTS["hx_last"]:
                        crosses(1, XC, XD)
                else:
                    region(0, SUPM, SDNM, XA, XB)
                    if halo_full:
                        region(1, SUPH, SDNH, XC, XD)
                ps = psnm
                ps_prev = psnm
                if t < time_steps and OPTS["no_copy"]:
                    pass
                elif t < time_steps:
                    # PSUM fp32 -> SBUF fp16 state copies (per region,
                    # halo first so its consumers unblock earlier)
                    hcopy = (V.tensor_copy if OPTS["copy_dve_halo"]
                             else AC.copy)
                    if t not in ex_ts:
                        if OPTS["halo_first"]:
                            hcopy(q16[cur][:, 1, 1:1023], psnh[:, 1:1023])
                            AC.copy(q16[cur][:, 0, 1:1023], psnm[:, 1:1023])
                        else:
                            AC.copy(q16[cur][:, 0, 1:1023], psnm[:, 1:1023])
                            hcopy(q16[cur][:, 1, 1:1023], psnh[:, 1:1023])
                    else:
                        AC.copy(q16[cur][:, 0, 1:1023], psnm[:, 1:1023])
                        # ---- halo exchange: AllGather of main-region q ----
                        bounce = dramp.tile([P, W], f16, tag="bounce",
                                            name=f"bounce_{t}")
                        gath = dramp.tile([n_cores * P, W], f16, tag="gath",
                                          addr_space="Shared",
                                          name=f"gath_{t}")
                        nc.sync.dma_start(out=bounce[:, :],
                                          in_=q16[cur][:, 0, 1:1023])
                        nc.gpsimd.collective_compute(
                            "AllGather", mybir.AluOpType.bypass,
                            ins=[bounce.opt()], outs=[gath.opt()],
                            replica_groups=[list(range(n_cores))],
                        )
                        stage = coef.tile([P, n_cores, W], f16,
                                          name=f"stage_{t}")
                        for b in range(n_cores):
                            nc.sync.dma_start(
                                out=stage[:, b, :],
                                in_=gath[b * P:(b + 1) * P, :])
                        for lo, hi in BANKS:
                            for b in range(n_cores):
                                mm(psnh[:, lo:hi],
                                   es[:, b * P:(b + 1) * P],
                                   stage[:, b, lo - 1:hi - 1],
                                   start=(b == 0), stop=(b == n_cores - 1))
                        AC.copy(q16[cur][:, 1, 1:1023], psnh[:, 1:1023])

            out_sb = coef.tile([P, W], f32, name="out_sb")
            nc.vector.tensor_mul(out_sb[:, :], ps[:, 1:1023], rsm[:, :])
            nc.sync.dma_start(out=pout_d[:, :], in_=out_sb[:, :])

    nc.finalize()
    return nc


def _get_nc(n_cores, time_steps, nx, ny):
    key = (n_cores, time_steps, nx, ny)
    if key not in _cached:
        _cached[key] = _build(n_cores, time_steps, nx, ny)
    return _cached[key]


def kernel(u, f, time_steps):
    from concourse.bass_utils import run_bass_kernel_spmd

    u = np.asarray(u)
    f = np.asarray(f)
    ts = int(time_steps)
    N = u.shape[0]
    nc = _get_nc(NCORES, ts, N, u.shape[1])
    in_maps = _host_inputs(u, f, NCORES, ts)
    res = run_bass_kernel_spmd(nc, in_maps, list(range(NCORES))).results
    interior = np.concatenate([r["pout"] for r in res], axis=0)
    h = 1.0 / (N - 1)
    xs = (np.arange(N, dtype=np.float64) * h).astype(np.float32)
    out = np.empty((N, N), dtype=np.float32)
    out[:, 1:N - 1] = interior
    out[:, 0] = xs
    out[:, N - 1] = 1.0 - xs
    return out
